# revision 2
# baseline (speedup 1.0000x reference)
"""Trainium2 8-core tensor-parallel causal attention layer (prefill, pos=0).

Sharding: heads split across 8 cores (2 heads each). Per core:
  1. Q^T/K^T (head-dim-major) and V (token-major) projections for its 2 heads
     from a host-transposed bf16 copy of h,
  2. RoPE via an even/odd head-dim permutation baked into Wq/Wk columns,
  3. causal attention in the transposed domain (scores^T = K^T_tile.T @ Q^T;
     exp without max-subtraction — scores are O(1); row-sums via ones-vector
     matmul; per-head normalization via gpsimd partition_broadcast),
  4. AllGather of per-head attention outputs (bf16, per batch), then a
     256-row slice of the output d-dimension with its Wo column slice.
Host-side: inputs transposed/sliced/cast bf16; outputs concatenated+transposed.
"""

import numpy as np
import ml_dtypes

import concourse.bass as bass
import concourse.tile as tile
from concourse import bacc, mybir
from concourse.bass_utils import run_bass_kernel_spmd

BF16 = mybir.dt.bfloat16
F32 = mybir.dt.float32
AF = mybir.ActivationFunctionType

B, S, D = 4, 2048, 2048
H, HD = 16, 128
NCORES = 8
HL = H // NCORES          # heads per core = 2
E = HL * HD               # per-core qkv width = 256
T = B * S                 # tokens = 8192
TT = 512                  # token tile (free dim)
NT_B = S // TT            # token tiles per batch = 4
DC = D // 128             # contraction chunks = 16
DS = D // NCORES          # output d-slice per core = 256
SCALE = 1.0 / np.sqrt(HD)

_cache = {}


def _build():
    nc = bacc.Bacc("TRN2", target_bir_lowering=False, debug=False,
                   num_devices=NCORES)

    hT_ext = nc.dram_tensor("hT", [D, T], BF16, kind="ExternalInput")
    wq_ext = nc.dram_tensor("wqT", [D, E], BF16, kind="ExternalInput")
    wk_ext = nc.dram_tensor("wkT", [D, E], BF16, kind="ExternalInput")
    wv_ext = nc.dram_tensor("wvT", [D, E], BF16, kind="ExternalInput")
    wo_ext = nc.dram_tensor("woT", [D, DS], BF16, kind="ExternalInput")
    cos_ext = nc.dram_tensor("cosT", [HD // 2, S], F32, kind="ExternalInput")
    sin_ext = nc.dram_tensor("sinT", [HD // 2, S], F32, kind="ExternalInput")
    mask_ext = nc.dram_tensor("maskT", [128, 4 * TT], BF16, kind="ExternalInput")
    out_ext = nc.dram_tensor("out", [DS, T], F32, kind="ExternalOutput")

    with tile.TileContext(nc) as tc:
        with (
            tc.tile_pool(name="weights", bufs=1) as wpool,
            tc.tile_pool(name="consts", bufs=1) as cpool,
            tc.tile_pool(name="ht", bufs=18) as htpool,
            tc.tile_pool(name="qkv", bufs=2) as qkvpool,
            tc.tile_pool(name="attn", bufs=4) as apool,
            tc.tile_pool(name="rtmp", bufs=6) as rpool,
            tc.tile_pool(name="small", bufs=3) as spool,
            tc.tile_pool(name="wor", bufs=20) as worpool,
            tc.tile_pool(name="ost", bufs=4) as ostpool,
            tc.tile_pool(name="ps", bufs=6, space="PSUM") as pspool,
            tc.tile_pool(name="psr", bufs=2, space="PSUM") as psrpool,
            tc.tile_pool(name="dram", bufs=2, space="DRAM") as dpool,
        ):
            # ---- persistent weights / constants -------------------------
            def load_w(ext, cols, tag):
                ts = []
                for dc in range(DC):
                    t = wpool.tile([128, cols], BF16, tag=f"{tag}{dc}",
                                   name=f"{tag}{dc}")
                    nc.sync.dma_start(t[:], ext.ap()[dc * 128:(dc + 1) * 128, :])
                    ts.append(t)
                return ts

            wq_sb = load_w(wq_ext, E, "wq")
            wk_sb = load_w(wk_ext, E, "wk")
            wv_sb = load_w(wv_ext, E, "wv")
            wo_sb = load_w(wo_ext, DS, "wo")

            cos_sb = cpool.tile([64, S], F32, tag="cos", name="cos")
            nc.sync.dma_start(cos_sb[:], cos_ext.ap())
            sin_sb = cpool.tile([64, S], F32, tag="sin", name="sin")
            nc.sync.dma_start(sin_sb[:], sin_ext.ap())
            mask_sb = cpool.tile([128, 4 * TT], BF16, tag="mask", name="mask")
            nc.sync.dma_start(mask_sb[:], mask_ext.ap())
            ones_sb = cpool.tile([128, 1], F32, tag="ones", name="ones")
            nc.vector.memset(ones_sb[:], 1.0)

            def proj(b):
                """QKV projections + RoPE for batch b."""
                qT = [qkvpool.tile([HD, S], BF16, tag=f"qT{lh}",
                                   name=f"qT{lh}_{b}") for lh in range(HL)]
                kT = [qkvpool.tile([HD, S], BF16, tag=f"kT{lh}",
                                   name=f"kT{lh}_{b}") for lh in range(HL)]
                v_sb = [qkvpool.tile([128, E], BF16, tag=f"v{vt}",
                                     name=f"v{vt}_{b}")
                        for vt in range(S // 128)]
                for tt in range(NT_B):
                    gt = NT_B * b + tt
                    ht = []
                    for dc in range(DC):
                        t = htpool.tile([128, TT], BF16, tag="ht",
                                        name=f"ht{dc}_{gt}")
                        nc.sync.dma_start(
                            t[:], hT_ext.ap()[dc * 128:(dc + 1) * 128,
                                              gt * TT:(gt + 1) * TT])
                        ht.append(t)
                    cs = cos_sb[:, tt * TT:(tt + 1) * TT]
                    sn = sin_sb[:, tt * TT:(tt + 1) * TT]
                    for w_sb, dstT in ((wq_sb, qT), (wk_sb, kT)):
                        for lh in range(HL):
                            ps = pspool.tile([128, TT], F32, tag="ps",
                                             name=f"psp{b}_{tt}_{lh}")
                            for dc in range(DC):
                                nc.tensor.matmul(
                                    ps[:],
                                    lhsT=w_sb[dc][:, lh * HD:(lh + 1) * HD],
                                    rhs=ht[dc][:],
                                    start=(dc == 0), stop=(dc == DC - 1))
                            # RoPE: rows 0:64 = even pairs (x0), 64:128 = odd (x1)
                            dst = dstT[lh][:, tt * TT:(tt + 1) * TT]
                            t1 = rpool.tile([64, TT], F32, tag="rtmp",
                                            name=f"t1_{b}{tt}{lh}")
                            t2 = rpool.tile([64, TT], F32, tag="rtmp",
                                            name=f"t2_{b}{tt}{lh}")
                            nc.vector.tensor_mul(t1[:], ps[0:64, :], cs)
                            nc.vector.tensor_mul(t2[:], ps[64:128, :], sn)
                            nc.vector.tensor_sub(dst[0:64, :], t1[:], t2[:])
                            t3 = rpool.tile([64, TT], F32, tag="rtmp",
                                            name=f"t3_{b}{tt}{lh}")
                            t4 = rpool.tile([64, TT], F32, tag="rtmp",
                                            name=f"t4_{b}{tt}{lh}")
                            nc.vector.tensor_mul(t3[:], ps[0:64, :], sn)
                            nc.vector.tensor_mul(t4[:], ps[64:128, :], cs)
                            nc.vector.tensor_add(dst[64:128, :], t3[:], t4[:])
                    for vt in range(TT // 128):
                        ps = pspool.tile([128, E], F32, tag="ps",
                                         name=f"psv{b}_{tt}_{vt}")
                        for dc in range(DC):
                            nc.tensor.matmul(
                                ps[:],
                                lhsT=ht[dc][:, vt * 128:(vt + 1) * 128],
                                rhs=wv_sb[dc][:],
                                start=(dc == 0), stop=(dc == DC - 1))
                        nc.vector.tensor_copy(v_sb[tt * 4 + vt][:], ps[:])
                return qT, kT, v_sb

            def attn(b, qT, kT, v_sb):
                """Causal attention, transposed domain; stages bf16 out^T
                into the per-batch AllGather bounce."""
                ag_in = dpool.tile([E, S], BF16, tag="ag_in", name=f"ag_in{b}")
                ag_out = dpool.tile([H * HD, S], BF16, tag="ag_out",
                                    name=f"ag_out{b}", addr_space="Shared")
                for lh in range(HL):
                    for qt in range(NT_B):
                        n_kt = (qt + 1) * (TT // 128)
                        Ssum = apool.tile([128, TT], F32, tag="S",
                                          name=f"S{b}{lh}{qt}")
                        aps = pspool.tile([128, TT], F32, tag="ps",
                                          name=f"aps{b}_{lh}_{qt}")

                        def attn_v(pp, pkt, stop):
                            nc.tensor.matmul(
                                aps[:],
                                lhsT=v_sb[pkt][:, lh * HD:(lh + 1) * HD],
                                rhs=pp[:],
                                start=(pkt == 0), stop=stop,
                                skip_group_check=True)

                        pend = []
                        for kt in range(n_kt):
                            sps = pspool.tile([128, TT], F32, tag="ps",
                                              name=f"sps{b}_{lh}_{qt}_{kt}")
                            nc.tensor.matmul(
                                sps[:],
                                lhsT=kT[lh][:, kt * 128:(kt + 1) * 128],
                                rhs=qT[lh][:, qt * TT:(qt + 1) * TT],
                                start=True, stop=True)
                            if len(pend) >= 2:
                                attn_v(*pend.pop(0), stop=False)
                            probs = apool.tile([128, TT], BF16, tag="probs",
                                               name=f"pr{b}_{lh}_{qt}_{kt}")
                            nc.scalar.activation(probs[:], sps[:], AF.Exp,
                                                 scale=float(SCALE))
                            diag = kt - qt * (TT // 128)
                            if diag >= 0:
                                nc.vector.tensor_mul(
                                    probs[:], probs[:],
                                    mask_sb[:, diag * TT:(diag + 1) * TT])
                            if kt == 0:
                                nc.vector.tensor_copy(Ssum[:], probs[:])
                            else:
                                nc.vector.tensor_add(Ssum[:], Ssum[:], probs[:])
                            pend.append((probs, kt))
                        while pend:
                            attn_v(*pend.pop(0), stop=(len(pend) == 0))

                        rps = psrpool.tile([1, TT], F32, tag="rs",
                                           name=f"rs{b}_{lh}_{qt}")
                        nc.tensor.matmul(rps[:], lhsT=ones_sb[:, 0:1],
                                         rhs=Ssum[:], start=True, stop=True)
                        recip = spool.tile([1, TT], F32, tag="recip",
                                           name=f"rc{b}{lh}{qt}")
                        nc.vector.reciprocal(recip[:], rps[:])
                        bcast = spool.tile([128, TT], F32, tag="bcast",
                                           name=f"bc{b}{lh}{qt}")
                        nc.gpsimd.partition_broadcast(bcast[:], recip[:])
                        agst = spool.tile([128, TT], BF16, tag="agst",
                                          name=f"ag{b}{lh}{qt}")
                        nc.vector.tensor_mul(agst[:], aps[:], bcast[:])
                        nc.sync.dma_start(
                            ag_in[lh * HD:(lh + 1) * HD,
                                  qt * TT:(qt + 1) * TT], agst[:])
                nc.gpsimd.collective_compute(
                    "AllGather", mybir.AluOpType.bypass,
                    ins=[ag_in[:].opt()],
                    outs=[ag_out[:].opt()],
                    replica_groups=[list(range(NCORES))])
                return ag_out

            def wo(b, ag_out):
                """Output projection: this core's 256-row d-slice, batch b."""
                for st in range(NT_B):
                    rts = []
                    for ec in range(DC):
                        t = worpool.tile([128, TT], BF16, tag="wor",
                                         name=f"wor{ec}_{b}{st}")
                        nc.sync.dma_start(
                            t[:], ag_out[ec * 128:(ec + 1) * 128,
                                          st * TT:(st + 1) * TT])
                        rts.append(t)
                    for m in range(DS // 128):
                        ps = pspool.tile([128, TT], F32, tag="ps",
                                         name=f"pso{b}_{st}_{m}")
                        for ec in range(DC):
                            nc.tensor.matmul(
                                ps[:],
                                lhsT=wo_sb[ec][:, m * 128:(m + 1) * 128],
                                rhs=rts[ec][:],
                                start=(ec == 0), stop=(ec == DC - 1))
                        ost = ostpool.tile([128, TT], F32, tag="ost",
                                           name=f"ost{b}{st}{m}")
                        nc.scalar.copy(ost[:], ps[:])
                        nc.sync.dma_start(
                            out_ext.ap()[m * 128:(m + 1) * 128,
                                         b * S + st * TT:b * S + (st + 1) * TT],
                            ost[:])

            pending_wo = []
            for b in range(B):
                q, k, v = proj(b)
                ag = attn(b, q, k, v)
                pending_wo.append((b, ag))
                if b >= 1:
                    wo(*pending_wo.pop(0))
            for args in pending_wo:
                wo(*args)

    nc.compile()
    return nc


def _prep_inputs(h, Wq, Wk, Wv, Wo, freqs_cos, freqs_sin):
    bf = ml_dtypes.bfloat16
    hT = np.ascontiguousarray(
        np.asarray(h, np.float32).transpose(2, 0, 1).reshape(D, T)).astype(bf)
    cosT = np.ascontiguousarray(np.asarray(freqs_cos, np.float32).T)
    sinT = np.ascontiguousarray(np.asarray(freqs_sin, np.float32).T)
    perm = np.concatenate([np.arange(0, HD, 2), np.arange(1, HD, 2)])
    p = np.arange(128)[:, None]
    j = np.arange(TT)[None, :]
    mask = np.concatenate(
        [(j >= 128 * i + p).astype(np.float32) for i in range(4)],
        axis=1).astype(bf)

    Wq = np.asarray(Wq, np.float32); Wk = np.asarray(Wk, np.float32)
    Wv = np.asarray(Wv, np.float32); Wo = np.asarray(Wo, np.float32)
    in_maps = []
    for g in range(NCORES):
        rows = slice(E * g, E * (g + 1))
        wq_s = Wq[rows, :].reshape(HL, HD, D)[:, perm, :].reshape(E, D)
        wk_s = Wk[rows, :].reshape(HL, HD, D)[:, perm, :].reshape(E, D)
        wv_s = Wv[rows, :]
        wo_s = Wo[DS * g:DS * (g + 1), :]
        in_maps.append({
            "hT": hT,
            "wqT": np.ascontiguousarray(wq_s.T).astype(bf),
            "wkT": np.ascontiguousarray(wk_s.T).astype(bf),
            "wvT": np.ascontiguousarray(wv_s.T).astype(bf),
            "woT": np.ascontiguousarray(wo_s.T).astype(bf),
            "cosT": cosT,
            "sinT": sinT,
            "maskT": np.ascontiguousarray(mask),
        })
    return in_maps


def _run(in_maps, **kw):
    if "nc" not in _cache:
        _cache["nc"] = _build()
    return run_bass_kernel_spmd(_cache["nc"], in_maps,
                                core_ids=list(range(NCORES)), **kw)


def kernel(h, Wq, Wk, Wv, Wo, K_cache=None, V_cache=None,
           freqs_cos=None, freqs_sin=None, pos=0, **_ignored):
    assert int(pos) == 0
    in_maps = _prep_inputs(h, Wq, Wk, Wv, Wo, freqs_cos, freqs_sin)
    res = _run(in_maps)
    fullT = np.concatenate(
        [np.asarray(res.results[g]["out"], np.float32) for g in range(NCORES)],
        axis=0)
    return np.ascontiguousarray(
        fullT.reshape(D, B, S).transpose(1, 2, 0)).astype(np.float32)
